# revision 16
# baseline (speedup 1.0000x reference)
"""Trainium2 Bass kernel for a 2-layer edge-weighted GraphSAGE network.

Strategy (8 NeuronCores, dst-sharded):
  * Host converts the edge list (src, dst, w) into the dense row-normalized
    adjacency operator A[d, s] = sum_e w_e / max(deg_d, 1), so each layer's
    weighted segment-mean becomes a dense matmul h_N = A @ h.
  * Nodes (rows of A) are sharded across the 8 cores: core c owns dst range
    [1250c, 1250(c+1)).  A^T is stored fp8e4m3 scaled by 64 (keeps entries in
    the fp8 normal range; the 1/64 is folded into the PSUM->SBUF copy), so
    the whole 12.5MB per-core shard is loaded once and stays resident in
    SBUF — layer 2 re-reads it for free.
  * Aggregations run transposed on the TensorEngine: features on PSUM
    partitions, local dst nodes on the free axis; fp16 stationary x fp8
    moving, f32 accumulate.
  * Layer-1 output x is produced twice: fp16 for the local linear path and
    fp8 for aggregation; the fp8 copy is PE-transposed and AllGathered in
    two column-halves so the second half's transfer hides under layer-2
    compute.  A tiny warm-up collective at kernel start absorbs the one-time
    collective rendezvous / launch-skew cost.
  * Measured end-to-end relative error vs the f32 reference: ~4e-3.
"""

import os
import sys
import types

sys.path.insert(0, "/opt/trn_rl_repo")

import numpy as np

import concourse.bacc as bacc
import concourse.tile as tile
from concourse import mybir
from concourse import bass_utils
from concourse.masks import make_identity

N_NODES = 10000
N_EDGES = 640000
D_IN, D_HID, D_OUT = 128, 256, 64
N_CORES = 8
P = 128
NB = N_NODES // N_CORES          # 1250 local dst nodes per core
KR = 79                          # real src k-blocks (ceil(10000/128))
KB = 80                          # padded to a multiple of the quad size
KQ = KB // 2                     # A^T stream pairs
NPAD = KB * P
ASCALE = 64.0                    # fp8 pre-scale on A (undone in ACT copies)
F8 = mybir.dt.float8e4
F16 = mybir.dt.float16
F32 = mybir.dt.float32

# free-axis chunks of the local dst range (PSUM bank = 512 f32)
N_CHUNKS = [(0, 512), (512, 1024), (1024, NB)]
DST_BLOCKS = [(b * P, min((b + 1) * P, NB)) for b in range((NB + P - 1) // P)]
XG = 8                           # x k-blocks per batched load

_compiled_nc = None
LAST_EXEC_NS = None


def _build_nc():
    nc = bacc.Bacc("TRN2", target_bir_lowering=False, debug=False,
                   num_devices=N_CORES)

    as_d = nc.dram_tensor("as8", [KQ, P, 2 * NB], F8, kind="ExternalInput")
    al_d = nc.dram_tensor("al8", [5, P, 2 * NB], F8, kind="ExternalInput")
    hl_d = nc.dram_tensor("hl", [P, 10 * D_IN], F16, kind="ExternalInput")
    hs_d = nc.dram_tensor("hsb", [P, KB * D_IN], F16, kind="ExternalInput")
    ht_d = nc.dram_tensor("ht", [D_IN, NB], F16, kind="ExternalInput")
    w1_d = nc.dram_tensor("w1", [2 * D_IN, D_HID], F16, kind="ExternalInput")
    w2_d = nc.dram_tensor("w2", [2 * D_HID, D_OUT], F16, kind="ExternalInput")
    b1_d = nc.dram_tensor("b1c", [P, 2], F32, kind="ExternalInput")
    b2_d = nc.dram_tensor("b2c", [D_OUT, 1], F32, kind="ExternalInput")
    out_d = nc.dram_tensor("outT", [D_OUT, NB], F32, kind="ExternalOutput")

    with tile.TileContext(nc) as tc:
        with (
            tc.tile_pool(name="const", bufs=1) as cpool,
            tc.tile_pool(name="acache", bufs=1) as acpool,
            tc.tile_pool(name="work", bufs=1) as wpool,
            tc.tile_pool(name="xstream", bufs=1) as xpool,
            tc.tile_pool(name="dram", bufs=1, space="DRAM") as dpool,
        ):
            # ---- warm-up collective: absorbs the one-time collective init /
            # cross-core launch-skew rendezvous in parallel with layer 1.
            warm_sb = cpool.tile([1, 16], F16)
            nc.vector.memset(warm_sb[:], 0.0)
            warm_in = dpool.tile([1, 16], F16)
            warm_out = dpool.tile([N_CORES, 16], F16, addr_space="Shared")
            nc.gpsimd.dma_start(out=warm_in[:], in_=warm_sb[:])
            nc.gpsimd.collective_compute(
                "AllGather", mybir.AluOpType.bypass,
                replica_groups=[list(range(N_CORES))],
                ins=[warm_in.opt()], outs=[warm_out.opt()])

            # ---- resident loads: h k-blocks (scalar ring) + full A^T (both) --
            hsb = cpool.tile([P, KB * D_IN], F16)
            HC = KB * D_IN // 4
            for j in range(4):
                nc.scalar.dma_start(out=hsb[:, j * HC:(j + 1) * HC],
                                    in_=hs_d[:, j * HC:(j + 1) * HC])
            acq = [acpool.tile([P, 2 * NB], F8, name=f"acq{q}")
                   for q in range(KQ)]
            for q in range(KQ):
                eng = nc.sync if q % 2 == 0 else nc.scalar
                eng.dma_start(out=acq[q][:], in_=as_d[q])

            aloc = [acpool.tile([P, 2 * NB], F8, name=f"al{lp}")
                    for lp in range(5)]
            for lp in range(5):
                nc.scalar.dma_start(out=aloc[lp][:], in_=al_d[lp])

            def art(k, n0, n1):
                return acq[k // 2][:, (k % 2) * NB + n0:(k % 2) * NB + n1]

            hls = cpool.tile([P, 10 * D_IN], F16)
            nc.scalar.dma_start(out=hls[:], in_=hl_d[:])
            hts = cpool.tile([P, NB], F16)
            nc.scalar.dma_start(out=hts[:], in_=ht_d[:])
            w1s = cpool.tile([P, 2 * D_HID], F16)
            for k in range(2):
                nc.scalar.dma_start(out=w1s[:, k * D_HID:(k + 1) * D_HID],
                                    in_=w1_d[k * P:(k + 1) * P, :])
            w2s = cpool.tile([P, 4 * D_OUT], F16)
            for k in range(4):
                nc.scalar.dma_start(out=w2s[:, k * D_OUT:(k + 1) * D_OUT],
                                    in_=w2_d[k * P:(k + 1) * P, :])
            b1s = cpool.tile([P, 2], F32)
            nc.scalar.dma_start(out=b1s[:], in_=b1_d[:])
            b2s = cpool.tile([D_OUT, 1], F32)
            nc.scalar.dma_start(out=b2s[:], in_=b2_d[:])
            ident = cpool.tile([P, P], F16)
            make_identity(nc, ident[:])

            hNT = wpool.tile([P, NB], F16)
            xT = [wpool.tile([P, NB], F16, name=f"xT{m}") for m in range(2)]
            xNT = [wpool.tile([P, NB], F16, name=f"xNT{m}") for m in range(2)]
            xloc8m = [wpool.tile([P, len(DST_BLOCKS) * P], F8, name=f"xloc8m{m}")
                      for m in range(2)]
            outsb = wpool.tile([D_OUT, NB], F32)

            # ---- layer 1 aggregation: hN^T = (1/64) sum_k hk^T . As_k -------
            with tc.tile_pool(name="ps1", bufs=1, space="PSUM") as ps1:
                hN_ps = ps1.tile([P, NB], F32, space="PSUM")
                # core-local src rows first (zeroed in as8; no DMA deps)
                for b in range(10):
                    for (n0, n1) in N_CHUNKS:
                        nc.tensor.matmul(
                            out=hN_ps[:, n0:n1],
                            lhsT=hls[:, b * D_IN:(b + 1) * D_IN],
                            rhs=aloc[b // 2][:, (b % 2) * NB + n0:(b % 2) * NB + n1],
                            start=(b == 0), stop=False)
                for k in range(KR):
                    for (n0, n1) in N_CHUNKS:
                        nc.tensor.matmul(out=hN_ps[:, n0:n1],
                                         lhsT=hsb[:, k * D_IN:(k + 1) * D_IN],
                                         rhs=art(k, n0, n1),
                                         start=False, stop=(k == KR - 1))
                nc.scalar.activation(out=hNT[:], in_=hN_ps[:],
                                     func=mybir.ActivationFunctionType.Copy,
                                     scale=1.0 / ASCALE)

            # ---- layer 1 linear: x^T = relu(W1^T . [h; hN]^T + b1) ----------
            # x is produced twice: fp16 for the local linear path, fp8 for
            # the aggregation/all-gather path.
            cat1 = [hts, hNT]
            with tc.tile_pool(name="ps2", bufs=1, space="PSUM") as ps2:
                y_ps = [ps2.tile([P, NB], F32, space="PSUM", name=f"y_ps{m}")
                        for m in range(2)]
                for m in range(2):
                    for (n0, n1) in N_CHUNKS:
                        for k in range(2):
                            nc.tensor.matmul(
                                out=y_ps[m][:, n0:n1],
                                lhsT=w1s[:, k * D_HID + m * P: k * D_HID + (m + 1) * P],
                                rhs=cat1[k][:, n0:n1],
                                start=(k == 0), stop=(k == 1))
                for m in range(2):
                    for (n0, n1) in N_CHUNKS:
                        nc.scalar.activation(
                            out=xT[m][:, n0:n1], in_=y_ps[m][:, n0:n1],
                            func=mybir.ActivationFunctionType.Relu,
                            bias=b1s[:, m:m + 1])

            # ---- transpose x8^T -> x8 (row-major local shard) ---------------
            lbw = DST_BLOCKS[-1][1] - DST_BLOCKS[-1][0]
            lb0 = (len(DST_BLOCKS) - 1) * P
            with tc.tile_pool(name="ps3", bufs=2, space="PSUM") as ps3:
                for m in range(2):          # m-major: half 0 fully first
                    # ragged last block: zero the lanes past the shard end so
                    # the local partial matmuls read zeros, not garbage
                    nc.vector.memset(xloc8m[m][:, lb0:lb0 + P], 0.0)
                    for b, (d0, d1) in enumerate(DST_BLOCKS):
                        bw = d1 - d0
                        tps = ps3.tile([P, P], F16, space="PSUM", name="tps")
                        nc.tensor.transpose(out=tps[:bw, :],
                                            in_=xT[m][:, d0:d1],
                                            identity=ident[:])
                        nc.vector.tensor_copy(
                            out=xloc8m[m][:bw, b * P:(b + 1) * P],
                            in_=tps[:bw, :])

            # ---- layer 2 aggregation: xN^T = (1/64) sum_k xk^T . As_k -------
            # The core-local 1/8 of the sum runs BEFORE the all-gather (its x
            # rows are local; those rows are zeroed out of as8 on the host),
            # filling the PE while the collective rendezvous completes.
            ps4_ctx = tc.tile_pool(name="ps4", bufs=1, space="PSUM")
            ps4 = ps4_ctx.__enter__()
            xN_ps = [ps4.tile([P, NB], F32, space="PSUM", name=f"xN_ps{m}")
                     for m in range(2)]
            for m in range(2):
                for lp in range(5):
                    lhs_pair = xloc8m[m][:, (2 * lp) * P:(2 * lp + 2) * P] \
                        .rearrange("p (two f) -> p two f", two=2)
                    rhs_pair = aloc[lp][:] \
                        .rearrange("p (two d) -> p two d", two=2)
                    for (n0, n1) in N_CHUNKS:
                        nc.tensor.matmul(
                            out=xN_ps[m][:, n0:n1],
                            lhsT=lhs_pair,
                            rhs=rhs_pair[:, :, n0:n1],
                            perf_mode=mybir.MatmulPerfMode.DoubleRow,
                            start=(lp == 0), stop=False)

            # ---- all-gather x8 across cores, split in column halves ---------
            ag_in = [dpool.tile([NB, P], F8, name=f"ag_in{m}") for m in range(2)]
            ag_out = [dpool.tile([N_NODES, P], F8, addr_space="Shared",
                                 name=f"ag_out{m}") for m in range(2)]
            for m in range(2):
                for b, (d0, d1) in enumerate(DST_BLOCKS):
                    bw = d1 - d0
                    nc.scalar.dma_start(
                        out=ag_in[m][d0:d1, :],
                        in_=xloc8m[m][:bw, b * P:(b + 1) * P])
            for m in range(2):
                nc.gpsimd.collective_compute(
                    "AllGather", mybir.AluOpType.bypass,
                    replica_groups=[list(range(N_CORES))],
                    ins=[ag_in[m].opt()], outs=[ag_out[m].opt()])

            if True:
                for m in range(2):
                    for g in range(0, KB, XG):
                        gk = min(XG, KB - g)
                        xq = xpool.tile([P, XG * P], F8, name=f"xq{m}", bufs=3)
                        full = min((g + gk) * P, (KR - 1) * P) - g * P
                        nc.scalar.dma_start(
                            out=xq[:, :full].rearrange("p (k f) -> p k f", f=P),
                            in_=ag_out[m][g * P: g * P + full, :]
                                .rearrange("(k p) f -> p k f", p=P))
                        if g + gk == KB:  # ragged block 78 + zero pad block 79
                            nc.vector.memset(xq[:, (gk - 2) * P:], 0.0)
                            tail = N_NODES - (KR - 1) * P
                            nc.scalar.dma_start(
                                out=xq[:tail, (gk - 2) * P: (gk - 1) * P],
                                in_=ag_out[m][(KR - 1) * P:, :])
                        # fp8 DoubleRow: contract k-block pairs, 2 MACs/cycle
                        for kp in range(g // 2, (g + gk) // 2):
                            kk2 = kp * 2 - g
                            lhs_pair = xq[:, kk2 * P:(kk2 + 2) * P] \
                                .rearrange("p (two f) -> p two f", two=2)
                            rhs_pair = acq[kp][:] \
                                .rearrange("p (two d) -> p two d", two=2)
                            for (n0, n1) in N_CHUNKS:
                                nc.tensor.matmul(
                                    out=xN_ps[m][:, n0:n1],
                                    lhsT=lhs_pair,
                                    rhs=rhs_pair[:, :, n0:n1],
                                    perf_mode=mybir.MatmulPerfMode.DoubleRow,
                                    start=False, stop=(kp == KB // 2 - 1))
                    nc.scalar.activation(out=xNT[m][:], in_=xN_ps[m][:],
                                         func=mybir.ActivationFunctionType.Copy,
                                         scale=1.0 / ASCALE)
            ps4_ctx.__exit__(None, None, None)

            # ---- layer 2 linear: out^T = W2^T . [x; xN]^T + b2 --------------
            cat2 = [xT[0], xT[1], xNT[0], xNT[1]]
            with tc.tile_pool(name="ps5", bufs=1, space="PSUM") as ps5:
                o_ps = ps5.tile([D_OUT, NB], F32, space="PSUM")
                for (n0, n1) in N_CHUNKS:
                    for k in range(4):
                        nc.tensor.matmul(
                            out=o_ps[:, n0:n1],
                            lhsT=w2s[:, k * D_OUT:(k + 1) * D_OUT],
                            rhs=cat2[k][:, n0:n1],
                            start=(k == 0), stop=(k == 3))
                nc.scalar.activation(out=outsb[:], in_=o_ps[:],
                                     func=mybir.ActivationFunctionType.Identity,
                                     bias=b2s[:, 0:1])
            nc.sync.dma_start(out=out_d[:], in_=outsb[:])

    nc.compile()
    return nc


def _get_nc():
    global _compiled_nc
    if _compiled_nc is None:
        _compiled_nc = _build_nc()
    return _compiled_nc


def _enable_profile_hook():
    """Register the NTFF profiling hook that trn_boot skips when the image's
    antenv lacks axon_hooks (profiling only; used when GNN_PROFILE=1)."""
    try:
        import antenv
        if "antenv.axon_hooks" not in sys.modules:
            mod = types.ModuleType("antenv.axon_hooks")
            _h = [None]
            mod.set_axon_ntff_profile_hook = lambda hook: _h.__setitem__(0, hook)
            mod.get_axon_ntff_profile_hook = lambda: _h[0]
            sys.modules["antenv.axon_hooks"] = mod
            antenv.axon_hooks = mod
        from trn_agent_boot.trn_boot import _ntff_profile_via_ctypes
        hook = _ntff_profile_via_ctypes("/opt/axon/libaxon_pjrt.so")
        if hook is not None:
            sys.modules["antenv.axon_hooks"].set_axon_ntff_profile_hook(hook)
            return True
    except Exception:
        pass
    return False


def _host_prep(h, w, src, dst, W1, b1, W2, b2):
    import ml_dtypes
    import scipy.sparse as sp
    deg = np.bincount(dst, minlength=N_NODES).astype(np.float32)
    w_norm = (w[:, 0] * (ASCALE / np.maximum(deg, 1.0)[dst])).astype(np.float32)
    # AT[s, d] = sum of scaled w_norm over edges (s -> d): 64*A^T
    AT = sp.coo_matrix((w_norm, (src, dst)), shape=(N_NODES, N_NODES)).toarray()
    AT8 = np.zeros((NPAD, N_NODES), dtype=ml_dtypes.float8_e4m3)
    AT8[:N_NODES] = AT
    hp = np.zeros((NPAD, D_IN), dtype=np.float16)
    hp[:N_NODES] = h
    # hsb[p, k*128+f] = h[k*128+p, f] (SBUF layout, contiguous per partition)
    hsb = np.ascontiguousarray(
        hp.reshape(KB, P, D_IN).transpose(1, 0, 2).reshape(P, KB * D_IN))

    w1c = W1.astype(np.float16)
    w2c = W2.astype(np.float16)
    b1c = np.ascontiguousarray(b1.reshape(2, P).T)
    b2c = b2.reshape(D_OUT, 1)

    in_maps = []
    for c in range(N_CORES):
        sl = slice(c * NB, (c + 1) * NB)
        ATc = AT8[:, sl].copy()
        # the core's own src rows are aggregated locally (al8) before the
        # all-gather; zero them in the main operator to avoid double counting
        al = np.zeros((5 * 2 * P, NB), dtype=ATc.dtype)
        al[:NB] = ATc[c * NB:(c + 1) * NB]
        ATc[c * NB:(c + 1) * NB] = 0
        # as8[q, p, j*NB+d] = ATc[(2q+j)*128+p, d] (pair-interleaved)
        as8 = np.ascontiguousarray(
            ATc.reshape(KQ, 2, P, NB).transpose(0, 2, 1, 3)
            .reshape(KQ, P, 2 * NB))
        al8 = np.ascontiguousarray(
            al.reshape(5, 2, P, NB).transpose(0, 2, 1, 3)
            .reshape(5, P, 2 * NB))
        hl = np.zeros((10 * P, D_IN), dtype=np.float16)
        hl[:NB] = h[sl]
        hlb = np.ascontiguousarray(
            hl.reshape(10, P, D_IN).transpose(1, 0, 2).reshape(P, 10 * D_IN))
        in_maps.append({
            "as8": as8,
            "al8": al8,
            "hl": hlb,
            "hsb": hsb,
            "ht": np.ascontiguousarray(h[sl].T.astype(np.float16)),
            "w1": w1c,
            "w2": w2c,
            "b1c": b1c,
            "b2c": b2c,
        })
    return in_maps


def kernel(h, w, src, dst, W1, b1, W2, b2):
    global LAST_EXEC_NS
    h = np.asarray(h, dtype=np.float32)
    w = np.asarray(w, dtype=np.float32)
    src = np.asarray(src)
    dst = np.asarray(dst)
    W1 = np.asarray(W1, dtype=np.float32)
    b1 = np.asarray(b1, dtype=np.float32)
    W2 = np.asarray(W2, dtype=np.float32)
    b2 = np.asarray(b2, dtype=np.float32)

    in_maps = _host_prep(h, w, src, dst, W1, b1, W2, b2)
    nc = _get_nc()
    trace = os.environ.get("GNN_PROFILE") == "1" and _enable_profile_hook()
    res = bass_utils.run_bass_kernel_spmd(
        nc, in_maps, core_ids=list(range(N_CORES)), trace=trace)
    LAST_EXEC_NS = res.exec_time_ns

    out = np.concatenate(
        [res.results[c]["outT"].T for c in range(N_CORES)], axis=0)
    return out.astype(np.float32)


# revision 17
# speedup vs baseline: 1.2339x; 1.2339x over previous
"""Trainium2 Bass kernel for a 2-layer edge-weighted GraphSAGE network.

Strategy (8 NeuronCores, dst-sharded):
  * Host converts the edge list (src, dst, w) into the dense row-normalized
    adjacency operator A[d, s] = sum_e w_e / max(deg_d, 1), so each layer's
    weighted segment-mean becomes a dense matmul h_N = A @ h.
  * Nodes (rows of A) are sharded across the 8 cores: core c owns dst range
    [1250c, 1250(c+1)).  A^T is stored fp8e4m3 scaled by 64 (keeps entries in
    the fp8 normal range; the 1/64 is folded into the PSUM->SBUF copy), so
    the whole 12.5MB per-core shard is loaded once and stays resident in
    SBUF — layer 2 re-reads it for free.
  * Aggregations run transposed on the TensorEngine: features on PSUM
    partitions, local dst nodes on the free axis; fp16 stationary x fp8
    moving, f32 accumulate.
  * Layer-1 output x is produced twice: fp16 for the local linear path and
    fp8 for aggregation; the fp8 copy is PE-transposed and AllGathered in
    two column-halves so the second half's transfer hides under layer-2
    compute.  A tiny warm-up collective at kernel start absorbs the one-time
    collective rendezvous / launch-skew cost.
  * Measured end-to-end relative error vs the f32 reference: ~4e-3.
"""

import os
import sys
import types

sys.path.insert(0, "/opt/trn_rl_repo")

import numpy as np

import concourse.bacc as bacc
import concourse.tile as tile
from concourse import mybir
from concourse import bass_utils
from concourse.masks import make_identity

N_NODES = 10000
N_EDGES = 640000
D_IN, D_HID, D_OUT = 128, 256, 64
N_CORES = 8
P = 128
NB = N_NODES // N_CORES          # 1250 local dst nodes per core
KR = 79                          # real src k-blocks (ceil(10000/128))
KB = 80                          # padded to a multiple of the quad size
KQ = KB // 2                     # A^T stream pairs
NPAD = KB * P
ASCALE = 64.0                    # fp8 pre-scale on A (undone in ACT copies)
F8 = mybir.dt.float8e4
F16 = mybir.dt.float16
F32 = mybir.dt.float32

# free-axis chunks of the local dst range (PSUM bank = 512 f32)
N_CHUNKS = [(0, 512), (512, 1024), (1024, NB)]
DST_BLOCKS = [(b * P, min((b + 1) * P, NB)) for b in range((NB + P - 1) // P)]
XG = 8                           # x k-blocks per batched load

_compiled_nc = None
LAST_EXEC_NS = None


def _build_nc():
    nc = bacc.Bacc("TRN2", target_bir_lowering=False, debug=False,
                   num_devices=N_CORES)

    as_d = nc.dram_tensor("as8", [KQ, P, 2 * NB], F8, kind="ExternalInput")
    al_d = nc.dram_tensor("al8", [5, P, 2 * NB], F8, kind="ExternalInput")
    hl_d = nc.dram_tensor("hl", [P, 10 * D_IN], F16, kind="ExternalInput")
    hs_d = nc.dram_tensor("hsb", [P, KB * D_IN], F16, kind="ExternalInput")
    ht_d = nc.dram_tensor("ht", [D_IN, NB], F16, kind="ExternalInput")
    w1_d = nc.dram_tensor("w1", [2 * D_IN, D_HID], F16, kind="ExternalInput")
    w2_d = nc.dram_tensor("w2", [2 * D_HID, D_OUT], F16, kind="ExternalInput")
    b1_d = nc.dram_tensor("b1c", [P, 2], F32, kind="ExternalInput")
    b2_d = nc.dram_tensor("b2c", [D_OUT, 1], F32, kind="ExternalInput")
    out_d = nc.dram_tensor("outT", [D_OUT, NB], F32, kind="ExternalOutput")

    with tile.TileContext(nc) as tc:
        with (
            tc.tile_pool(name="const", bufs=1) as cpool,
            tc.tile_pool(name="acache", bufs=1) as acpool,
            tc.tile_pool(name="work", bufs=1) as wpool,
            tc.tile_pool(name="xstream", bufs=1) as xpool,
            tc.tile_pool(name="dram", bufs=1, space="DRAM") as dpool,
        ):
            # ---- warm-up collective: absorbs the one-time collective init /
            # cross-core launch-skew rendezvous in parallel with layer 1.
            warm_sb = cpool.tile([1, 16], F16)
            nc.vector.memset(warm_sb[:], 0.0)
            warm_in = dpool.tile([1, 16], F16)
            warm_out = dpool.tile([N_CORES, 16], F16, addr_space="Shared")
            nc.gpsimd.dma_start(out=warm_in[:], in_=warm_sb[:])
            nc.gpsimd.collective_compute(
                "AllGather", mybir.AluOpType.bypass,
                replica_groups=[list(range(N_CORES))],
                ins=[warm_in.opt()], outs=[warm_out.opt()])

            # ---- resident loads: h k-blocks (scalar ring) + full A^T (both) --
            hsb = cpool.tile([P, KB * D_IN], F16)
            HC = KB * D_IN // 4
            for j in range(4):
                nc.scalar.dma_start(out=hsb[:, j * HC:(j + 1) * HC],
                                    in_=hs_d[:, j * HC:(j + 1) * HC])
            acq = [acpool.tile([P, 2 * NB], F8, name=f"acq{q}")
                   for q in range(KQ)]
            for q in range(KQ):
                eng = nc.sync if q % 2 == 0 else nc.scalar
                eng.dma_start(out=acq[q][:], in_=as_d[q])

            aloc = [acpool.tile([P, 2 * NB], F8, name=f"al{lp}")
                    for lp in range(5)]
            for lp in range(5):
                nc.scalar.dma_start(out=aloc[lp][:], in_=al_d[lp])

            def art(k, n0, n1):
                return acq[k // 2][:, (k % 2) * NB + n0:(k % 2) * NB + n1]

            hls = cpool.tile([P, 10 * D_IN], F16)
            nc.scalar.dma_start(out=hls[:], in_=hl_d[:])
            hts = cpool.tile([P, NB], F16)
            nc.scalar.dma_start(out=hts[:], in_=ht_d[:])
            w1s = cpool.tile([P, 2 * D_HID], F16)
            for k in range(2):
                nc.scalar.dma_start(out=w1s[:, k * D_HID:(k + 1) * D_HID],
                                    in_=w1_d[k * P:(k + 1) * P, :])
            w2s = cpool.tile([P, 4 * D_OUT], F16)
            for k in range(4):
                nc.scalar.dma_start(out=w2s[:, k * D_OUT:(k + 1) * D_OUT],
                                    in_=w2_d[k * P:(k + 1) * P, :])
            b1s = cpool.tile([P, 2], F32)
            nc.scalar.dma_start(out=b1s[:], in_=b1_d[:])
            b2s = cpool.tile([D_OUT, 1], F32)
            nc.scalar.dma_start(out=b2s[:], in_=b2_d[:])
            ident = cpool.tile([P, P], F16)
            make_identity(nc, ident[:])

            hNT = wpool.tile([P, NB], F16)
            xT = [wpool.tile([P, NB], F16, name=f"xT{m}") for m in range(2)]
            xNT = [wpool.tile([P, NB], F16, name=f"xNT{m}") for m in range(2)]
            xloc8m = [wpool.tile([P, len(DST_BLOCKS) * P], F8, name=f"xloc8m{m}")
                      for m in range(2)]
            outsb = wpool.tile([D_OUT, NB], F32)

            # ---- layer 1 aggregation: hN^T = (1/64) sum_k hk^T . As_k -------
            with tc.tile_pool(name="ps1", bufs=1, space="PSUM") as ps1:
                hN_ps = ps1.tile([P, NB], F32, space="PSUM")
                for k in range(KR):
                    for (n0, n1) in N_CHUNKS:
                        nc.tensor.matmul(out=hN_ps[:, n0:n1],
                                         lhsT=hsb[:, k * D_IN:(k + 1) * D_IN],
                                         rhs=art(k, n0, n1),
                                         start=(k == 0), stop=False)
                # core-local src rows last (zeroed in as8); aloc has arrived
                # long before the main sweep finishes
                for b in range(10):
                    for (n0, n1) in N_CHUNKS:
                        nc.tensor.matmul(
                            out=hN_ps[:, n0:n1],
                            lhsT=hls[:, b * D_IN:(b + 1) * D_IN],
                            rhs=aloc[b // 2][:, (b % 2) * NB + n0:(b % 2) * NB + n1],
                            start=False, stop=(b == 9))
                nc.scalar.activation(out=hNT[:], in_=hN_ps[:],
                                     func=mybir.ActivationFunctionType.Copy,
                                     scale=1.0 / ASCALE)

            # ---- layer 1 linear: x^T = relu(W1^T . [h; hN]^T + b1) ----------
            # x is produced twice: fp16 for the local linear path, fp8 for
            # the aggregation/all-gather path.
            cat1 = [hts, hNT]
            with tc.tile_pool(name="ps2", bufs=1, space="PSUM") as ps2:
                y_ps = [ps2.tile([P, NB], F32, space="PSUM", name=f"y_ps{m}")
                        for m in range(2)]
                for m in range(2):
                    for (n0, n1) in N_CHUNKS:
                        for k in range(2):
                            nc.tensor.matmul(
                                out=y_ps[m][:, n0:n1],
                                lhsT=w1s[:, k * D_HID + m * P: k * D_HID + (m + 1) * P],
                                rhs=cat1[k][:, n0:n1],
                                start=(k == 0), stop=(k == 1))
                for m in range(2):
                    for (n0, n1) in N_CHUNKS:
                        nc.scalar.activation(
                            out=xT[m][:, n0:n1], in_=y_ps[m][:, n0:n1],
                            func=mybir.ActivationFunctionType.Relu,
                            bias=b1s[:, m:m + 1])

            # ---- transpose x8^T -> x8 (row-major local shard) ---------------
            lbw = DST_BLOCKS[-1][1] - DST_BLOCKS[-1][0]
            lb0 = (len(DST_BLOCKS) - 1) * P
            with tc.tile_pool(name="ps3", bufs=2, space="PSUM") as ps3:
                for m in range(2):          # m-major: half 0 fully first
                    # ragged last block: zero the lanes past the shard end so
                    # the local partial matmuls read zeros, not garbage
                    nc.vector.memset(xloc8m[m][:, lb0:lb0 + P], 0.0)
                    for b, (d0, d1) in enumerate(DST_BLOCKS):
                        bw = d1 - d0
                        tps = ps3.tile([P, P], F16, space="PSUM", name="tps")
                        nc.tensor.transpose(out=tps[:bw, :],
                                            in_=xT[m][:, d0:d1],
                                            identity=ident[:])
                        nc.vector.tensor_copy(
                            out=xloc8m[m][:bw, b * P:(b + 1) * P],
                            in_=tps[:bw, :])

            # ---- layer 2 aggregation: xN^T = (1/64) sum_k xk^T . As_k -------
            # The core-local 1/8 of the sum runs BEFORE the all-gather (its x
            # rows are local; those rows are zeroed out of as8 on the host),
            # filling the PE while the collective rendezvous completes.
            ps4_ctx = tc.tile_pool(name="ps4", bufs=1, space="PSUM")
            ps4 = ps4_ctx.__enter__()
            xN_ps = [ps4.tile([P, NB], F32, space="PSUM", name=f"xN_ps{m}")
                     for m in range(2)]
            for m in range(2):
                for lp in range(5):
                    lhs_pair = xloc8m[m][:, (2 * lp) * P:(2 * lp + 2) * P] \
                        .rearrange("p (two f) -> p two f", two=2)
                    rhs_pair = aloc[lp][:] \
                        .rearrange("p (two d) -> p two d", two=2)
                    for (n0, n1) in N_CHUNKS:
                        nc.tensor.matmul(
                            out=xN_ps[m][:, n0:n1],
                            lhsT=lhs_pair,
                            rhs=rhs_pair[:, :, n0:n1],
                            perf_mode=mybir.MatmulPerfMode.DoubleRow,
                            start=(lp == 0), stop=False)

            # ---- all-gather x8 across cores, split in column halves ---------
            ag_in = [dpool.tile([NB, P], F8, name=f"ag_in{m}") for m in range(2)]
            ag_out = [dpool.tile([N_NODES, P], F8, addr_space="Shared",
                                 name=f"ag_out{m}") for m in range(2)]
            for m in range(2):
                for b, (d0, d1) in enumerate(DST_BLOCKS):
                    bw = d1 - d0
                    nc.scalar.dma_start(
                        out=ag_in[m][d0:d1, :],
                        in_=xloc8m[m][:bw, b * P:(b + 1) * P])
            for m in range(2):
                nc.gpsimd.collective_compute(
                    "AllGather", mybir.AluOpType.bypass,
                    replica_groups=[list(range(N_CORES))],
                    ins=[ag_in[m].opt()], outs=[ag_out[m].opt()])

            if True:
                for m in range(2):
                    for g in range(0, KB, XG):
                        gk = min(XG, KB - g)
                        xq = xpool.tile([P, XG * P], F8, name=f"xq{m}", bufs=3)
                        full = min((g + gk) * P, (KR - 1) * P) - g * P
                        nc.scalar.dma_start(
                            out=xq[:, :full].rearrange("p (k f) -> p k f", f=P),
                            in_=ag_out[m][g * P: g * P + full, :]
                                .rearrange("(k p) f -> p k f", p=P))
                        if g + gk == KB:  # ragged block 78 + zero pad block 79
                            nc.vector.memset(xq[:, (gk - 2) * P:], 0.0)
                            tail = N_NODES - (KR - 1) * P
                            nc.scalar.dma_start(
                                out=xq[:tail, (gk - 2) * P: (gk - 1) * P],
                                in_=ag_out[m][(KR - 1) * P:, :])
                        # fp8 DoubleRow: contract k-block pairs, 2 MACs/cycle
                        for kp in range(g // 2, (g + gk) // 2):
                            kk2 = kp * 2 - g
                            lhs_pair = xq[:, kk2 * P:(kk2 + 2) * P] \
                                .rearrange("p (two f) -> p two f", two=2)
                            rhs_pair = acq[kp][:] \
                                .rearrange("p (two d) -> p two d", two=2)
                            for (n0, n1) in N_CHUNKS:
                                nc.tensor.matmul(
                                    out=xN_ps[m][:, n0:n1],
                                    lhsT=lhs_pair,
                                    rhs=rhs_pair[:, :, n0:n1],
                                    perf_mode=mybir.MatmulPerfMode.DoubleRow,
                                    start=False, stop=(kp == KB // 2 - 1))
                    nc.scalar.activation(out=xNT[m][:], in_=xN_ps[m][:],
                                         func=mybir.ActivationFunctionType.Copy,
                                         scale=1.0 / ASCALE)
            ps4_ctx.__exit__(None, None, None)

            # ---- layer 2 linear: out^T = W2^T . [x; xN]^T + b2 --------------
            cat2 = [xT[0], xT[1], xNT[0], xNT[1]]
            with tc.tile_pool(name="ps5", bufs=1, space="PSUM") as ps5:
                o_ps = ps5.tile([D_OUT, NB], F32, space="PSUM")
                for (n0, n1) in N_CHUNKS:
                    for k in range(4):
                        nc.tensor.matmul(
                            out=o_ps[:, n0:n1],
                            lhsT=w2s[:, k * D_OUT:(k + 1) * D_OUT],
                            rhs=cat2[k][:, n0:n1],
                            start=(k == 0), stop=(k == 3))
                nc.scalar.activation(out=outsb[:], in_=o_ps[:],
                                     func=mybir.ActivationFunctionType.Identity,
                                     bias=b2s[:, 0:1])
            nc.sync.dma_start(out=out_d[:], in_=outsb[:])

    nc.compile()
    return nc


def _get_nc():
    global _compiled_nc
    if _compiled_nc is None:
        _compiled_nc = _build_nc()
    return _compiled_nc


def _enable_profile_hook():
    """Register the NTFF profiling hook that trn_boot skips when the image's
    antenv lacks axon_hooks (profiling only; used when GNN_PROFILE=1)."""
    try:
        import antenv
        if "antenv.axon_hooks" not in sys.modules:
            mod = types.ModuleType("antenv.axon_hooks")
            _h = [None]
            mod.set_axon_ntff_profile_hook = lambda hook: _h.__setitem__(0, hook)
            mod.get_axon_ntff_profile_hook = lambda: _h[0]
            sys.modules["antenv.axon_hooks"] = mod
            antenv.axon_hooks = mod
        from trn_agent_boot.trn_boot import _ntff_profile_via_ctypes
        hook = _ntff_profile_via_ctypes("/opt/axon/libaxon_pjrt.so")
        if hook is not None:
            sys.modules["antenv.axon_hooks"].set_axon_ntff_profile_hook(hook)
            return True
    except Exception:
        pass
    return False


def _host_prep(h, w, src, dst, W1, b1, W2, b2):
    import ml_dtypes
    import scipy.sparse as sp
    deg = np.bincount(dst, minlength=N_NODES).astype(np.float32)
    w_norm = (w[:, 0] * (ASCALE / np.maximum(deg, 1.0)[dst])).astype(np.float32)
    # AT[s, d] = sum of scaled w_norm over edges (s -> d): 64*A^T
    AT = sp.coo_matrix((w_norm, (src, dst)), shape=(N_NODES, N_NODES)).toarray()
    AT8 = np.zeros((NPAD, N_NODES), dtype=ml_dtypes.float8_e4m3)
    AT8[:N_NODES] = AT
    hp = np.zeros((NPAD, D_IN), dtype=np.float16)
    hp[:N_NODES] = h
    # hsb[p, k*128+f] = h[k*128+p, f] (SBUF layout, contiguous per partition)
    hsb = np.ascontiguousarray(
        hp.reshape(KB, P, D_IN).transpose(1, 0, 2).reshape(P, KB * D_IN))

    w1c = W1.astype(np.float16)
    w2c = W2.astype(np.float16)
    b1c = np.ascontiguousarray(b1.reshape(2, P).T)
    b2c = b2.reshape(D_OUT, 1)

    in_maps = []
    for c in range(N_CORES):
        sl = slice(c * NB, (c + 1) * NB)
        ATc = AT8[:, sl].copy()
        # the core's own src rows are aggregated locally (al8) before the
        # all-gather; zero them in the main operator to avoid double counting
        al = np.zeros((5 * 2 * P, NB), dtype=ATc.dtype)
        al[:NB] = ATc[c * NB:(c + 1) * NB]
        ATc[c * NB:(c + 1) * NB] = 0
        # as8[q, p, j*NB+d] = ATc[(2q+j)*128+p, d] (pair-interleaved)
        as8 = np.ascontiguousarray(
            ATc.reshape(KQ, 2, P, NB).transpose(0, 2, 1, 3)
            .reshape(KQ, P, 2 * NB))
        al8 = np.ascontiguousarray(
            al.reshape(5, 2, P, NB).transpose(0, 2, 1, 3)
            .reshape(5, P, 2 * NB))
        hl = np.zeros((10 * P, D_IN), dtype=np.float16)
        hl[:NB] = h[sl]
        hlb = np.ascontiguousarray(
            hl.reshape(10, P, D_IN).transpose(1, 0, 2).reshape(P, 10 * D_IN))
        in_maps.append({
            "as8": as8,
            "al8": al8,
            "hl": hlb,
            "hsb": hsb,
            "ht": np.ascontiguousarray(h[sl].T.astype(np.float16)),
            "w1": w1c,
            "w2": w2c,
            "b1c": b1c,
            "b2c": b2c,
        })
    return in_maps


def kernel(h, w, src, dst, W1, b1, W2, b2):
    global LAST_EXEC_NS
    h = np.asarray(h, dtype=np.float32)
    w = np.asarray(w, dtype=np.float32)
    src = np.asarray(src)
    dst = np.asarray(dst)
    W1 = np.asarray(W1, dtype=np.float32)
    b1 = np.asarray(b1, dtype=np.float32)
    W2 = np.asarray(W2, dtype=np.float32)
    b2 = np.asarray(b2, dtype=np.float32)

    in_maps = _host_prep(h, w, src, dst, W1, b1, W2, b2)
    nc = _get_nc()
    trace = os.environ.get("GNN_PROFILE") == "1" and _enable_profile_hook()
    res = bass_utils.run_bass_kernel_spmd(
        nc, in_maps, core_ids=list(range(N_CORES)), trace=trace)
    LAST_EXEC_NS = res.exec_time_ns

    out = np.concatenate(
        [res.results[c]["outT"].T for c in range(N_CORES)], axis=0)
    return out.astype(np.float32)
